# revision 1
# baseline (speedup 1.0000x reference)
"""Trainium2 Bass kernel for nn_BackpropKalmanFilter.

Math: after the Riccati recursion converges (t >= T1, ~320 steps), the Kalman
gain is constant and the filter is the LTI recursion
    x_t = A x_{t-1} + K z_t,   A = (I - K H) F  (contractive)
Since ||A^d|| < 1e-9 for d >= D (=512), each output is a finite convolution
over the last D measurements.  Blocked form with block L=32, J = D/L = 16:
    U_k   = C Z_k                        (local block response, C: 16 x 256)
    Y_k   = G Z_k + sum_{j=1..J} (P A^{L(j-1)}) U_{k-j}
          = W [Z_k ; U_{k-1} ; ... ; U_{k-J}]     (W: 512 x 512)
which is one dense matmul per block -- fully parallel.  T is sharded over
8 cores with a J-block halo; the pre-convergence transient (t < T0 ~ 832)
is computed exactly on the host and overwrites the first rows.
"""
import os
import sys

import numpy as np

sys.path.insert(0, "/opt/trn_rl_repo")
sys.path.insert(0, "/root/.axon_site")
sys.path.insert(0, "/root/.axon_site/_ro/pypackages")

N, M = 16, 8          # state / measurement dims
T = 500_000
L = 32                # block length
J = 16                # halo blocks (D = J*L = 512 decay length)
D = J * L
NCORES = 8
KB = 1960                         # blocks per core (padded: 4 stripes of 490)
TTOT = NCORES * KB * L            # 500224 padded steps
KC = KB + J                       # 1970 columns incl. halo
DTYPE_MODE = os.environ.get("KAL_DTYPE", "fp32")   # fp32 | bf16
REPS = int(os.environ.get("KAL_REPS", "1"))        # timing amplification
HWLOOP = int(os.environ.get("KAL_HWLOOP", "0"))    # hardware-loop reps (timing)

_cache = {}


# ----------------------------------------------------------------- host math
def _riccati(F, H, Q, R):
    F64, H64 = F.astype(np.float64), H.astype(np.float64)
    Q64, R64 = Q.astype(np.float64), R.astype(np.float64)
    P = np.eye(N)
    prevK = None
    T1 = None
    for t in range(4096):
        P = F64 @ P @ F64.T + Q64
        S = H64 @ P @ H64.T + R64
        K = P @ H64.T @ np.linalg.inv(S)
        P = (np.eye(N) - K @ H64) @ P
        if prevK is not None and T1 is None and np.abs(K - prevK).max() < 1e-13:
            T1 = t
        prevK = K
    assert T1 is not None
    A = (np.eye(N) - K @ H64) @ F64
    return K, A, T1


def _build_weights(F, H, Q, R):
    K_ss, A, T1 = _riccati(F, H, Q, R)
    npow = L * J + 2
    Apow = np.empty((npow, N, N))
    Apow[0] = np.eye(N)
    for i in range(1, npow):
        Apow[i] = Apow[i - 1] @ A
    AK = Apow @ K_ss                                   # A^d K

    C = np.concatenate([AK[L - 1 - j] for j in range(L)], axis=1)  # (16, 256)

    G = np.zeros((N * L, M * L))
    for i in range(L):
        for j in range(i + 1):
            G[i * N:(i + 1) * N, j * M:(j + 1) * M] = AK[i - j]
    Pm = np.concatenate([Apow[i + 1] for i in range(L)], axis=0)   # (512, 16)
    # j order matches the us-tile row layout: us0 rows hold j=8..1 (descending),
    # us1 rows hold j=16..9, so the pass-B shift DMA has all-positive strides
    jorder = list(range(8, 0, -1)) + list(range(16, 8, -1))
    PB = np.concatenate([Pm @ Apow[L * (j - 1)] for j in jorder],
                        axis=1)                                    # (512, 256)
    W = np.concatenate([G, PB], axis=1)                            # (512, 512)
    T0 = ((T1 + D) + L - 1) // L * L                               # 832
    return W, C, T0


def _host_transient(meas, F, H, Q, R, T0):
    F64, H64 = F.astype(np.float64), H.astype(np.float64)
    Q64, R64 = Q.astype(np.float64), R.astype(np.float64)
    x = np.zeros(N)
    P = np.eye(N)
    out = np.empty((T0, N))
    for t in range(T0):
        x = F64 @ x
        P = F64 @ P @ F64.T + Q64
        z = meas[t, :, 0].astype(np.float64)
        S = H64 @ P @ H64.T + R64
        K = P @ H64.T @ np.linalg.inv(S)
        x = x + K @ (z - H64 @ x)
        P = (np.eye(N) - K @ H64) @ P
        out[t] = x
    return out


def _chunks(total, step=512):
    out = []
    s = 0
    while s < total:
        out.append((s, min(step, total - s)))
        s += step
    return out


# ------------------------------------------------------------- device program
def _build_program(dtype_mode):
    import concourse.bacc as bacc
    import concourse.bass as bass
    import concourse.tile as tile
    from concourse import mybir

    f32 = mybir.dt.float32
    cdt = {"bf16": mybir.dt.bfloat16,
           "fp32r": mybir.dt.float32r}.get(dtype_mode, f32)

    nc = bacc.Bacc("TRN2", target_bir_lowering=False, debug=False,
                   enable_asserts=False, num_devices=NCORES)

    zmat_d = nc.dram_tensor("zmat", [2, 128, KC], cdt, kind="ExternalInput").ap()
    wT_d = nc.dram_tensor("wT", [4, 128, 512], cdt, kind="ExternalInput").ap()
    cT_d = nc.dram_tensor("cT", [2, 128, 128], cdt, kind="ExternalInput").ap()
    gidx_d = nc.dram_tensor("gidx", [128, 32], mybir.dt.int16,
                            kind="ExternalInput").ap()
    out_d = nc.dram_tensor("out", [4, 128, KB], f32, kind="ExternalOutput").ap()

    from concourse import library_config
    SW = 490                      # stripe width (4 stripes of 490 = KB)
    NI = 512                      # gather num_idxs (>= SW + 8, mult of 16)
    with tile.TileContext(nc, trace_sim=False) as tc:
        with (
            tc.tile_pool(name="const", bufs=1) as const,
            tc.tile_pool(name="zms", bufs=3) as zmsp,
            tc.tile_pool(name="up", bufs=3) as up,
            tc.tile_pool(name="usp", bufs=3) as usp,
            tc.tile_pool(name="ysp", bufs=6) as ysp,
            tc.tile_pool(name="psA", bufs=2, space=bass.MemorySpace.PSUM) as psA,
            tc.tile_pool(name="psC", bufs=6, space=bass.MemorySpace.PSUM) as psC,
        ):
            nc.gpsimd.load_library(library_config.ap_gather)
            wt = [const.tile([128, 512], cdt, name=f"wt{i}") for i in range(4)]
            ct = [const.tile([128, 128], cdt, name=f"ct{i}") for i in range(2)]
            gix = const.tile([128, 32], mybir.dt.int16, name="gix")
            for i in range(4):
                nc.sync.dma_start(wt[i][:], wT_d[i])
            for i in range(2):
                nc.sync.dma_start(ct[i][:], cT_d[i])
            nc.sync.dma_start(gix[:], gidx_d[:])

            import contextlib
            loop_cm = tc.For_i(0, HWLOOP, 1) if HWLOOP else contextlib.nullcontext()
            with loop_cm:
                for rep in range(REPS):
                    for (s, w) in _chunks(KB, SW):
                        wj = w + J
                        zm = [zmsp.tile([128, KC], cdt, name=f"zm{i}",
                                        tag=f"zm{i}") for i in range(2)]
                        for i in range(2):
                            nc.sync.dma_start(zm[i][:, :wj],
                                              zmat_d[i][:, s:s + wj])
                        # Pass A: Urep = [C;..;C] @ Z  (U replicated in all
                        # 8 16-row groups), cols s .. s+w+J
                        urep = up.tile([128, KC], f32, name="urep", tag="urep")
                        pu = psA.tile([128, 512], f32, name="pu")
                        nc.tensor.matmul(pu[:, :wj], ct[0][:], zm[0][:, :wj],
                                         start=True, stop=False)
                        nc.tensor.matmul(pu[:, :wj], ct[1][:], zm[1][:, :wj],
                                         start=False, stop=True)
                        nc.vector.tensor_copy(urep[:, :wj], pu[:, :wj])
                        # Pass B: per-group shift via gpsimd gather:
                        # ust[16q+t, k] = urep[16q+t, q+k] = U[t, q+k]
                        ust = usp.tile([128, NI], f32, name="ust", tag="ust")
                        nc.gpsimd.ap_gather(ust[:, :NI], urep[:, :wj],
                                            gix[:, :NI // 16],
                                            channels=128, num_elems=wj,
                                            d=1, num_idxs=NI)
                        ustr = usp.tile([128, NI], cdt, name="ustr",
                                        tag="ustr")
                        nc.vector.tensor_copy(ustr[:, :w + 8], ust[:, :w + 8])
                        # Pass C: Y = W [Z ; Ushift]; Kt2 reads ust at +8
                        for mt in range(4):
                            py = psC.tile([128, 512], f32, name="py")
                            ms = slice(mt * 128, mt * 128 + 128)
                            nc.tensor.matmul(py[:, :w], wt[0][:, ms],
                                             zm[0][:, J:J + w],
                                             start=True, stop=False)
                            nc.tensor.matmul(py[:, :w], wt[1][:, ms],
                                             zm[1][:, J:J + w],
                                             start=False, stop=False)
                            nc.tensor.matmul(py[:, :w], wt[2][:, ms],
                                             ustr[:, 8:8 + w],
                                             start=False, stop=False)
                            nc.tensor.matmul(py[:, :w], wt[3][:, ms],
                                             ustr[:, :w],
                                             start=False, stop=True)
                            ysb = ysp.tile([128, 512], f32, name="ysb",
                                           tag="ysb")
                            if mt % 2 == 0:
                                nc.vector.tensor_copy(ysb[:, :w], py[:, :w])
                            else:
                                nc.scalar.copy(ysb[:, :w], py[:, :w])
                            eng = nc.gpsimd if mt % 2 == 0 else nc.scalar
                            eng.dma_start(out_d[mt][:, s:s + w],
                                          ysb[:, :w])
    nc.compile()
    return nc


# ------------------------------------------------------------------ interface
def _prepare(measurements, F, H, Q, R, dtype_mode):
    W, C, T0 = _build_weights(F, H, Q, R)
    np_dt = np.float32
    if dtype_mode == "bf16":
        import ml_dtypes
        np_dt = ml_dtypes.bfloat16

    wT = np.ascontiguousarray(W.T.astype(np.float32).reshape(4, 128, 512)).astype(np_dt)
    cT = np.ascontiguousarray(
        np.tile(C.T.astype(np.float32).reshape(2, 128, 16), (1, 1, 8))).astype(np_dt)
    SW, NI = 490, 512
    gidx = np.zeros((128, 32), np.int16)
    for q in range(8):
        for i in range(NI):
            v = i + q if i < SW + 8 else 0
            gidx[16 * q + i % 16, i // 16] = v

    meas_pad = np.zeros((TTOT, M), np.float32)
    meas_pad[:T] = measurements[:, :, 0]
    blocks = meas_pad.reshape(TTOT // L, 2, 128)      # (Ktot, ktile, 128)

    in_maps = []
    for c in range(NCORES):
        k0 = c * KB
        zc = np.zeros((2, 128, KC), np.float32)
        lo = max(0, k0 - J)
        src = blocks[lo:k0 + KB].transpose(1, 2, 0)   # (2,128,ncols)
        zc[:, :, J - (k0 - lo):] = src
        in_maps.append({"zmat": np.ascontiguousarray(zc).astype(np_dt),
                        "wT": wT, "cT": cT, "gidx": gidx})
    return in_maps, T0


def _assemble(results, meas, F, H, Q, R, T0):
    chunks = []
    for c in range(NCORES):
        o = results[c]["out"]                         # (4,128,KB) f32
        Y = o.reshape(512, KB)
        chunks.append(np.ascontiguousarray(Y.T).reshape(KB * L, N))
    full = np.concatenate(chunks, axis=0)[:T]
    full[:T0] = _host_transient(meas, F, H, Q, R, T0).astype(np.float32)
    return np.ascontiguousarray(full).reshape(T, N, 1).astype(np.float32)


def run(measurements, F, H, Q, R, trace=False):
    """Returns (output, BassKernelResults)."""
    from concourse.bass_utils import run_bass_kernel_spmd

    dtype_mode = DTYPE_MODE
    if "nc" not in _cache or _cache.get("mode") != dtype_mode:
        _cache["nc"] = _build_program(dtype_mode)
        _cache["mode"] = dtype_mode
    nc = _cache["nc"]
    in_maps, T0 = _prepare(measurements, F, H, Q, R, dtype_mode)
    res = run_bass_kernel_spmd(nc, in_maps, core_ids=list(range(NCORES)),
                               trace=trace)
    out = _assemble(res.results, measurements, F, H, Q, R, T0)
    return out, res


def kernel(measurements, F, H, Q, R):
    measurements = np.asarray(measurements, dtype=np.float32)
    F = np.asarray(F, dtype=np.float32)
    H = np.asarray(H, dtype=np.float32)
    Q = np.asarray(Q, dtype=np.float32)
    R = np.asarray(R, dtype=np.float32)
    out, _ = run(measurements, F, H, Q, R, trace=False)
    return out



# revision 2
# speedup vs baseline: 2.3263x; 2.3263x over previous
"""Trainium2 Bass kernel for nn_BackpropKalmanFilter.

Math: after the Riccati recursion converges (t >= T1, ~320 steps), the Kalman
gain is constant and the filter is the LTI recursion
    x_t = A x_{t-1} + K z_t,   A = (I - K H) F  (contractive)
||A^d|| < 6e-5 for d >= D (=256), so each output is a finite convolution over
the last D measurements.  Blocked form with block L=32, J = D/L = 8:
    U_k = C Z_k                     (block response, C: 16 x 256)
    X_k = sum_{j=0..7} A32^j U_{k-j}   (block-end state, A32 = A^32)
    Y_k = G Z_k + Pm X_{k-1}
All per-block; T is sharded over 8 cores with a J-block halo; the
pre-convergence transient (t < T0) is computed exactly on the host and
overwrites the first rows.

Two device mechanisms for the U-history combination (KAL_SHIFT):
  'mm'  - X via 8 column-offset matmuls accumulating in PSUM (default)
  'dma' - build a shifted replicated tile with 8 small SBUF->SBUF DMAs,
          then one 128-contraction matmul per output row tile
"""
import os
import sys

import numpy as np

sys.path.insert(0, "/opt/trn_rl_repo")
sys.path.insert(0, "/root/.axon_site")
sys.path.insert(0, "/root/.axon_site/_ro/pypackages")

N, M = 16, 8          # state / measurement dims
T = 500_000
L = 32                # block length
J = 8                 # halo blocks (D = J*L = 256 decay length)
D = J * L
NCORES = 8
KB = 1960                         # blocks per core (4 stripes of 490)
TTOT = NCORES * KB * L            # 501760 padded steps
KC = KB + J                       # 1968 columns incl. halo
SW = 490                          # stripe width
DTYPE_MODE = os.environ.get("KAL_DTYPE", "bf16")   # bf16 | fp32
OUT_MODE = os.environ.get("KAL_OUT", "bf16")       # bf16 | fp32
SHIFT_MODE = os.environ.get("KAL_SHIFT", "mm")     # mm | dma
REPS = int(os.environ.get("KAL_REPS", "1"))        # timing amplification
HWLOOP = int(os.environ.get("KAL_HWLOOP", "0"))    # hardware-loop reps

_cache = {}


# ----------------------------------------------------------------- host math
def _riccati(F, H, Q, R):
    F64, H64 = F.astype(np.float64), H.astype(np.float64)
    Q64, R64 = Q.astype(np.float64), R.astype(np.float64)
    P = np.eye(N)
    prevK = None
    T1 = None
    for t in range(1024):
        P = F64 @ P @ F64.T + Q64
        S = H64 @ P @ H64.T + R64
        K = P @ H64.T @ np.linalg.inv(S)
        P = (np.eye(N) - K @ H64) @ P
        if prevK is not None and T1 is None and np.abs(K - prevK).max() < 1e-13:
            T1 = t
        prevK = K
    assert T1 is not None
    A = (np.eye(N) - K @ H64) @ F64
    return K, A, T1


def _build_weights(F, H, Q, R):
    K_ss, A, T1 = _riccati(F, H, Q, R)
    npow = L * J + 2
    Apow = np.empty((npow, N, N))
    Apow[0] = np.eye(N)
    for i in range(1, npow):
        Apow[i] = Apow[i - 1] @ A
    AK = Apow @ K_ss                                   # A^d K

    C = np.concatenate([AK[L - 1 - j] for j in range(L)], axis=1)  # (16, 256)

    G = np.zeros((N * L, M * L))
    for i in range(L):
        for j in range(i + 1):
            G[i * N:(i + 1) * N, j * M:(j + 1) * M] = AK[i - j]
    Pm = np.concatenate([Apow[i + 1] for i in range(L)], axis=0)   # (512, 16)
    # dma variant: ust group q holds U_{k-(8-q)}  ->  PB col group q = Pm A^{32(7-q)}
    PB = np.concatenate([Pm @ Apow[L * (7 - q)] for q in range(J)],
                        axis=1)                                    # (512, 128)
    # mm variant lhsT packs
    a32T = np.concatenate([Apow[L * jj].T for jj in range(J)], axis=1)  # (16,128)
    pmT = Pm.T                                                     # (16, 512)
    T0 = ((T1 + D) + L - 1) // L * L
    return C, G, PB, a32T, pmT, T0


def _host_transient(meas, F, H, Q, R, T0):
    F64, H64 = F.astype(np.float64), H.astype(np.float64)
    Q64, R64 = Q.astype(np.float64), R.astype(np.float64)
    x = np.zeros(N)
    P = np.eye(N)
    out = np.empty((T0, N))
    for t in range(T0):
        x = F64 @ x
        P = F64 @ P @ F64.T + Q64
        z = meas[t, :, 0].astype(np.float64)
        S = H64 @ P @ H64.T + R64
        K = P @ H64.T @ np.linalg.inv(S)
        x = x + K @ (z - H64 @ x)
        P = (np.eye(N) - K @ H64) @ P
        out[t] = x
    return out


def _chunks(total, step=SW):
    out = []
    s = 0
    while s < total:
        out.append((s, min(step, total - s)))
        s += step
    return out


# ------------------------------------------------------------- device program
def _build_program(dtype_mode, out_mode, shift_mode):
    import concourse.bacc as bacc
    import concourse.bass as bass
    import concourse.tile as tile
    from concourse import mybir

    f32 = mybir.dt.float32
    cdt = mybir.dt.bfloat16 if dtype_mode == "bf16" else f32
    odt = mybir.dt.bfloat16 if out_mode == "bf16" else f32

    nc = bacc.Bacc("TRN2", target_bir_lowering=False, debug=False,
                   enable_asserts=False, num_devices=NCORES)

    zmat_d = nc.dram_tensor("zmat", [2, 128, KC], cdt, kind="ExternalInput").ap()
    wg_d = nc.dram_tensor("wg", [2, 128, 512], cdt, kind="ExternalInput").ap()
    wu_d = nc.dram_tensor("wu", [128, 512], cdt, kind="ExternalInput").ap()
    ct_d = nc.dram_tensor("ct", [2, 128, 16], cdt, kind="ExternalInput").ap()
    a32_d = nc.dram_tensor("a32", [16, 128], cdt, kind="ExternalInput").ap()
    pm_d = nc.dram_tensor("pm", [16, 512], cdt, kind="ExternalInput").ap()
    out_d = nc.dram_tensor("out", [4, 128, KB], odt, kind="ExternalOutput").ap()

    with tile.TileContext(nc, trace_sim=False) as tc:
        with (
            tc.tile_pool(name="const", bufs=1) as const,
            tc.tile_pool(name="up", bufs=2) as up,
            tc.tile_pool(name="vp", bufs=2) as vp,
            tc.tile_pool(name="usp", bufs=2) as usp,
            tc.tile_pool(name="ysp", bufs=6) as ysp,
            tc.tile_pool(name="psA", bufs=2, space=bass.MemorySpace.PSUM) as psA,
            tc.tile_pool(name="psV", bufs=2, space=bass.MemorySpace.PSUM) as psV,
            tc.tile_pool(name="psC", bufs=4, space=bass.MemorySpace.PSUM) as psC,
        ):
            zm = [const.tile([128, KC], cdt, name=f"zm{i}") for i in range(2)]
            wg = [const.tile([128, 512], cdt, name=f"wg{i}") for i in range(2)]
            wu = const.tile([128, 512], cdt, name="wu")
            ct2 = [const.tile([128, 16], cdt, name=f"ct{i}") for i in range(2)]
            a32 = const.tile([16, 128], cdt, name="a32")
            pm = const.tile([16, 512], cdt, name="pm")
            for i in range(2):
                nc.sync.dma_start(zm[i][:], zmat_d[i])
                nc.scalar.dma_start(wg[i][:], wg_d[i])
                nc.scalar.dma_start(ct2[i][:], ct_d[i])
            nc.sync.dma_start(wu[:], wu_d)
            nc.scalar.dma_start(a32[:], a32_d)
            nc.sync.dma_start(pm[:], pm_d)

            import contextlib
            loop_cm = tc.For_i(0, HWLOOP, 1) if HWLOOP else contextlib.nullcontext()
            with loop_cm:
                for rep in range(REPS):
                    for (s, w) in _chunks(KB):
                        wj = w + J
                        # Pass A: U_k = C Z_k for blocks s-8 .. s+w-1
                        pu = psA.tile([16, 512], f32, name="pu")
                        nc.tensor.matmul(pu[:, :wj], ct2[0][:], zm[0][:, s:s + wj],
                                         start=True, stop=False)
                        nc.tensor.matmul(pu[:, :wj], ct2[1][:], zm[1][:, s:s + wj],
                                         start=False, stop=True)
                        u16 = up.tile([16, 512], cdt, name="u16")
                        nc.vector.tensor_copy(u16[:, :wj], pu[:, :wj])

                        if shift_mode == "mm":
                            # X_b for b = s-1 .. s+w-1 (w+1 cols);
                            # X_b = sum_j A32^j U_{b-j}; U_b at u16 col b-s+8
                            pv = psV.tile([16, 512], f32, name="pv")
                            for jj in range(J):
                                nc.tensor.matmul(
                                    pv[:, :w + 1],
                                    a32[:, 16 * jj:16 * jj + 16],
                                    u16[:, 7 - jj:7 - jj + w + 1],
                                    start=(jj == 0), stop=(jj == J - 1))
                            v16 = vp.tile([16, 512], cdt, name="v16")
                            nc.scalar.copy(v16[:, :w + 1], pv[:, :w + 1])
                        else:
                            # shifted replicated tile: ust[16q+t, c] = u16[t, c+q]
                            ust = usp.tile([128, 512], cdt, name="ust")
                            for q in range(J):
                                eng = nc.sync if q % 2 == 0 else nc.scalar
                                eng.dma_start(ust[16 * q:16 * q + 16, :w],
                                              u16[0:16, q:q + w])

                        # Pass C: Y = G Z_k + Pm X_{k-1}
                        for mt in range(4):
                            py = psC.tile([128, 512], f32, name="py")
                            ms = slice(mt * 128, mt * 128 + 128)
                            nc.tensor.matmul(py[:, :w], wg[0][:, ms],
                                             zm[0][:, s + J:s + J + w],
                                             start=True, stop=False)
                            if mt >= 2:
                                nc.tensor.matmul(py[:, :w], wg[1][:, ms],
                                                 zm[1][:, s + J:s + J + w],
                                                 start=False, stop=False)
                            if shift_mode == "mm":
                                nc.tensor.matmul(py[:, :w], pm[:, ms],
                                                 v16[:, :w],
                                                 start=False, stop=True)
                            else:
                                nc.tensor.matmul(py[:, :w], wu[:, ms],
                                                 ust[:, :w],
                                                 start=False, stop=True)
                            ysb = ysp.tile([128, 512], odt, name="ysb")
                            if mt % 2 == 0:
                                nc.vector.tensor_copy(ysb[:, :w], py[:, :w])
                            else:
                                nc.scalar.copy(ysb[:, :w], py[:, :w])
                            eng = nc.sync if mt % 2 == 0 else nc.gpsimd
                            eng.dma_start(out_d[mt][:, s:s + w], ysb[:, :w])
    nc.compile()
    return nc


# ------------------------------------------------------------------ interface
def _np_dt(mode):
    if mode == "bf16":
        import ml_dtypes
        return ml_dtypes.bfloat16
    return np.float32


def _prepare(measurements, F, H, Q, R, dtype_mode):
    C, G, PB, a32T, pmT, T0 = _build_weights(F, H, Q, R)
    np_dt = _np_dt(dtype_mode)

    wg = np.ascontiguousarray(
        G.T.astype(np.float32).reshape(2, 128, 512)).astype(np_dt)
    wu = np.ascontiguousarray(PB.T.astype(np.float32)).astype(np_dt)
    ct = np.ascontiguousarray(
        C.T.astype(np.float32).reshape(2, 128, 16)).astype(np_dt)
    a32 = np.ascontiguousarray(a32T.astype(np.float32)).astype(np_dt)
    pm = np.ascontiguousarray(pmT.astype(np.float32)).astype(np_dt)

    meas_pad = np.zeros((TTOT, M), np.float32)
    meas_pad[:T] = measurements[:, :, 0]
    blocks = meas_pad.reshape(TTOT // L, 2, 128)      # (Ktot, ktile, 128)

    in_maps = []
    for c in range(NCORES):
        k0 = c * KB
        zc = np.zeros((2, 128, KC), np.float32)
        lo = max(0, k0 - J)
        src = blocks[lo:k0 + KB].transpose(1, 2, 0)   # (2,128,ncols)
        zc[:, :, J - (k0 - lo):] = src
        in_maps.append({"zmat": np.ascontiguousarray(zc).astype(np_dt),
                        "wg": wg, "wu": wu, "ct": ct, "a32": a32, "pm": pm})
    return in_maps, T0


def _assemble(results, meas, F, H, Q, R, T0):
    chunks = []
    for c in range(NCORES):
        o = np.asarray(results[c]["out"]).astype(np.float32)  # (4,128,KB)
        Y = o.reshape(512, KB)
        chunks.append(np.ascontiguousarray(Y.T).reshape(KB * L, N))
    full = np.concatenate(chunks, axis=0)[:T]
    full[:T0] = _host_transient(meas, F, H, Q, R, T0).astype(np.float32)
    return np.ascontiguousarray(full).reshape(T, N, 1).astype(np.float32)


def run(measurements, F, H, Q, R, trace=False):
    """Returns (output, BassKernelResults)."""
    from concourse.bass_utils import run_bass_kernel_spmd

    key = (DTYPE_MODE, OUT_MODE, SHIFT_MODE)
    if _cache.get("key") != key:
        _cache["nc"] = _build_program(*key)
        _cache["key"] = key
    nc = _cache["nc"]
    in_maps, T0 = _prepare(measurements, F, H, Q, R, DTYPE_MODE)
    res = run_bass_kernel_spmd(nc, in_maps, core_ids=list(range(NCORES)),
                               trace=trace)
    out = _assemble(res.results, measurements, F, H, Q, R, T0)
    return out, res


def kernel(measurements, F, H, Q, R):
    measurements = np.asarray(measurements, dtype=np.float32)
    F = np.asarray(F, dtype=np.float32)
    H = np.asarray(H, dtype=np.float32)
    Q = np.asarray(Q, dtype=np.float32)
    R = np.asarray(R, dtype=np.float32)
    out, _ = run(measurements, F, H, Q, R, trace=False)
    return out
